# revision 24
# baseline (speedup 1.0000x reference)
"""Bahdanau attention Trainium2 kernel.

Reference (per batch b):
  query  = Wq @ dh[b]                      # [HID]
  keys   = enc[b] @ Wk.T                   # [S, HID]
  energy = tanh(query + keys) @ v          # [S]
  energy = where(mask==0, -1e10, energy)
  attn   = softmax(energy)                 # [S]
  ctx    = attn @ enc[b]                   # [ENC]

Sharding: data-parallel over batch across 8 NeuronCores (8 batches each),
weights replicated.  Per core, per batch:
  - DMA enc[b] s-tile [128, 2048] fp32, cast to bf16 (DVE)
  - PE-transpose bf16 s-tile blocks into ET [e-part, s]  (keys matmul needs
    the contraction dim e on partitions)
  - keys^T [h,s] = WkT.T @ ET in bf16, accumulated fp32 in PSUM
  - tanh(+query bias) on ACT -> bf16
  - energy = v.T @ tanh  (PE, v stationary)
  - softmax on a [1, S] row (DVE/ACT)
  - ctx = attn.T @ enc_bf (PE, attn stationary), PSUM -> DRAM
"""

import numpy as np

import concourse.bass as bass
import concourse.bacc as bacc
import concourse.bass_isa as bass_isa
import concourse.mybir as mybir
import concourse.tile as tile
from concourse.masks import make_identity

B, S, HID, ENC = 64, 1024, 1024, 2048
NCORES = 8
BLOC = B // NCORES  # batches per core
P = 128
ST = S // P     # 8 s-tiles
HT = HID // P   # 8 h-tiles
ET = ENC // P   # 16 e-tiles
DT = HID // P   # 8 d-tiles
SC = S // 512   # 2 s-chunks of 512
EC = ENC // 512  # 4 e-chunks of 512

F32 = mybir.dt.float32
BF16 = mybir.dt.bfloat16
I32 = mybir.dt.int32
AX = mybir.AxisListType
AF = mybir.ActivationFunctionType
ALU = mybir.AluOpType

NEG = -1.0e10


def _emit(tc: tile.TileContext, nc: bass.Bass, dh, enc, mask, wq, wk, vv,
          ctx_out, attn_out, dbg=None):
    singles = tc.alloc_tile_pool(name="singles", bufs=1)
    wkT = singles.tile([P, ET, HID], BF16, tag="wkT")
    q_all = singles.tile([P, HT, BLOC], F32, tag="q_all")
    v_bf = singles.tile([P, HT], BF16, tag="v_bf")
    id_bf = singles.tile([P, P], BF16, tag="id_bf")
    negT = singles.tile([P, ST], F32, tag="negT")
    nc.vector.memset(negT, NEG)

    # ---- PSUM pools (7 banks total) ----
    tp_pool = tc.alloc_tile_pool(name="tp", bufs=2, space="PSUM")
    kp_pool = tc.alloc_tile_pool(name="kp", bufs=2, space="PSUM")
    eps_pool = tc.alloc_tile_pool(name="eps", bufs=1, space="PSUM")
    ctx_pool = tc.alloc_tile_pool(name="ctxp", bufs=2, space="PSUM")

    # ---------------- prep phase ----------------
    with tc.tile_pool(name="prep", bufs=2) as prep, \
         tc.tile_pool(name="prep1", bufs=1) as prep1:
        id32 = prep1.tile([P, P], F32, tag="id32")
        make_identity(nc, id32)
        nc.vector.tensor_copy(out=id_bf, in_=id32)

        # v -> [hp, ht] column layout, bf16
        v_col32 = prep1.tile([P, HT], F32, tag="v_col32")
        nc.sync.dma_start(out=v_col32, in_=vv[:].rearrange("(t p) -> p t", p=P))
        nc.vector.tensor_copy(out=v_bf, in_=v_col32)

        # Wk -> WkT bf16 via PE transpose of natural-layout h-tiles
        for ht in range(HT):
            wk32 = prep.tile([P, ENC], F32, tag="wk32")
            nc.sync.dma_start(out=wk32, in_=wk[ht * P:(ht + 1) * P, :])
            wkb = prep.tile([P, ENC], BF16, tag="wkb")
            nc.vector.tensor_copy(out=wkb, in_=wk32)
            for etg in range(ET // 4):
                tp = tp_pool.tile([P, 4, P], BF16, tag="tp")
                for j in range(4):
                    e0 = (etg * 4 + j) * P
                    nc.tensor.transpose(tp[:, j, :], wkb[:, e0:e0 + P], id_bf)
                nc.scalar.copy(
                    out=wkT[:, etg * 4:(etg + 1) * 4, ht * P:(ht + 1) * P],
                    in_=tp)

        # Wq -> WqT bf16 (transient), dh -> dhT bf16, then query matmul
        wqT = prep1.tile([P, DT, HID], BF16, tag="wqT")
        for ht in range(HT):
            wq32 = prep.tile([P, HID], F32, tag="wq32")
            nc.sync.dma_start(out=wq32, in_=wq[ht * P:(ht + 1) * P, :])
            wqb = prep.tile([P, HID], BF16, tag="wqb")
            nc.vector.tensor_copy(out=wqb, in_=wq32)
            for dtg in range(DT // 4):
                tp = tp_pool.tile([P, 4, P], BF16, tag="tp")
                for j in range(4):
                    d0 = (dtg * 4 + j) * P
                    nc.tensor.transpose(tp[:, j, :], wqb[:, d0:d0 + P], id_bf)
                nc.scalar.copy(
                    out=wqT[:, dtg * 4:(dtg + 1) * 4, ht * P:(ht + 1) * P],
                    in_=tp)

        dh32 = prep1.tile([P, HID], F32, tag="dh32")
        nc.vector.memset(dh32, 0.0)
        nc.sync.dma_start(out=dh32[:BLOC, :], in_=dh[:, :])
        dhb = prep1.tile([P, HID], BF16, tag="dhb")
        nc.vector.tensor_copy(out=dhb, in_=dh32)
        dhT = prep1.tile([P, DT, BLOC], BF16, tag="dhT")
        for dtg in range(DT // 4):
            tp = tp_pool.tile([P, 4, P], BF16, tag="tp")
            for j in range(4):
                d0 = (dtg * 4 + j) * P
                nc.tensor.transpose(tp[:, j, :], dhb[:, d0:d0 + P], id_bf)
            nc.scalar.copy(out=dhT[:, dtg * 4:(dtg + 1) * 4, :],
                                  in_=tp[:, :, :BLOC])

        for ht in range(HT):
            qp = tp_pool.tile([P, 4, P], F32, tag="tp")
            for dt in range(DT):
                nc.tensor.matmul(qp[:, 0, :BLOC],
                                 lhsT=wqT[:, dt, ht * P:(ht + 1) * P],
                                 rhs=dhT[:, dt, :],
                                 start=(dt == 0), stop=(dt == DT - 1))
            nc.scalar.copy(out=q_all[:, ht, :], in_=qp[:, 0, :BLOC])
        if dbg is not None:
            nc.sync.dma_start(out=dbg["q_all"][:], in_=q_all)

    # ---------------- main loop ----------------
    e32_pool = tc.alloc_tile_pool(name="e32", bufs=3)
    ebf_pool = tc.alloc_tile_pool(name="ebf", bufs=2)
    eT_pool = tc.alloc_tile_pool(name="eT", bufs=1)
    th_pool = tc.alloc_tile_pool(name="th", bufs=2)
    row_pool = tc.alloc_tile_pool(name="rows", bufs=2)

    for b in range(BLOC):
        # transposed inverted mask [s-part, s-tile] (1 where mask==0)
        maskT32 = row_pool.tile([P, ST], I32, tag="maskT32")
        nc.sync.dma_start(out=maskT32,
                          in_=mask[b, :].rearrange("(t p) -> p t", p=P))
        invT = row_pool.tile([P, ST], I32, tag="invT")
        nc.vector.tensor_scalar(out=invT, in0=maskT32, scalar1=0,
                                scalar2=None, op0=ALU.is_equal)

        # load + cast + transpose encoder s-tiles
        ebf = ebf_pool.tile([P, ST, ENC], BF16, tag="ebf")
        eT = eT_pool.tile([P, ET, S], BF16, tag="eT")
        for st in range(ST):
            e32 = e32_pool.tile([P, ENC], F32, tag="e32")
            nc.sync.dma_start(out=e32, in_=enc[b, st * P:(st + 1) * P, :])
            nc.vector.tensor_copy(out=ebf[:, st, :], in_=e32)
            for etg in range(ET // 4):
                tp = tp_pool.tile([P, 4, P], BF16, tag="tp")
                for j in range(4):
                    e0 = (etg * 4 + j) * P
                    nc.tensor.transpose(tp[:, j, :], ebf[:, st, e0:e0 + P],
                                        id_bf)
                nc.scalar.copy(
                    out=eT[:, etg * 4:(etg + 1) * 4, st * P:(st + 1) * P],
                    in_=tp)

        # keys + tanh + transposed energy
        eps = eps_pool.tile([P, ST], F32, tag="eps")
        for sc in range(SC):
            th = th_pool.tile([P, HT, 512], BF16, tag="th")
            for ht in range(HT):
                kp = kp_pool.tile([P, 512], F32, tag="kp")
                for et in range(ET):
                    nc.tensor.matmul(kp,
                                     lhsT=wkT[:, et, ht * P:(ht + 1) * P],
                                     rhs=eT[:, et, sc * 512:(sc + 1) * 512],
                                     start=(et == 0), stop=(et == ET - 1))
                nc.scalar.activation(th[:, ht, :], kp, AF.Tanh,
                                     bias=q_all[:, ht, b:b + 1], scale=1.0)
            for sb in range(4):
                stile = sc * 4 + sb
                for ht in range(HT):
                    nc.tensor.matmul(eps[:, stile:stile + 1],
                                     lhsT=th[:, ht, sb * P:(sb + 1) * P],
                                     rhs=v_bf[:, ht:ht + 1],
                                     start=(ht == 0), stop=(ht == HT - 1))
            if dbg is not None and b == 0:
                nc.sync.dma_start(out=dbg["th"][:, sc, :, :], in_=th)

        # energyT [s-part, s-tile] + mask
        energyT = row_pool.tile([P, ST], F32, tag="energyT")
        nc.vector.tensor_copy(out=energyT, in_=eps)
        nc.vector.copy_predicated(energyT, invT, negT)
        if dbg is not None:
            nc.sync.dma_start(
                out=dbg["energy"][b, :].rearrange("(t p) -> p t", p=P),
                in_=energyT)

        # softmax over all S: free-dim reduce + cross-partition reduce
        rmax8 = row_pool.tile([P, 1], F32, tag="rmax8")
        nc.vector.reduce_max(out=rmax8, in_=energyT, axis=AX.X)
        rmax = row_pool.tile([P, 1], F32, tag="rmax")
        nc.gpsimd.partition_all_reduce(rmax, rmax8, channels=P,
                                       reduce_op=bass_isa.ReduceOp.max)
        negmax = row_pool.tile([P, 1], F32, tag="negmax")
        nc.vector.tensor_scalar_mul(negmax, rmax, -1.0)
        pT = row_pool.tile([P, ST], F32, tag="pT")
        sum8 = row_pool.tile([P, 1], F32, tag="sum8")
        nc.scalar.activation(pT, energyT, AF.Exp, bias=negmax, scale=1.0,
                             accum_out=sum8)
        sumexp = row_pool.tile([P, 1], F32, tag="sumexp")
        nc.gpsimd.partition_all_reduce(sumexp, sum8, channels=P,
                                       reduce_op=bass_isa.ReduceOp.add)
        rinv = row_pool.tile([P, 1], F32, tag="rinv")
        nc.vector.reciprocal(out=rinv, in_=sumexp)
        attn_f = row_pool.tile([P, ST], F32, tag="attn_f")
        nc.vector.tensor_scalar_mul(attn_f, pT, rinv)
        attn_T = row_pool.tile([P, ST], BF16, tag="attn_T")
        nc.vector.tensor_scalar_mul(attn_T, pT, rinv)
        nc.sync.dma_start(
            out=attn_out[b, :].rearrange("(t p) -> p t", p=P), in_=attn_f)
        if dbg is not None:
            nc.sync.dma_start(out=dbg["sumexp"][b:b + 1, 0:1],
                              in_=sumexp[0:1, :])
            nc.sync.dma_start(out=dbg["sumexp"][b:b + 1, 1:2],
                              in_=rmax[0:1, :])

        # context
        ctx_row = row_pool.tile([1, ENC], F32, tag="ctx_row")
        for ec in range(EC):
            cps = ctx_pool.tile([1, 512], F32, tag="cps")
            for st in range(ST):
                nc.tensor.matmul(cps,
                                 lhsT=attn_T[:, st:st + 1],
                                 rhs=ebf[:, st, ec * 512:(ec + 1) * 512],
                                 start=(st == 0), stop=(st == ST - 1))
            nc.vector.tensor_copy(out=ctx_row[:, ec * 512:(ec + 1) * 512],
                                  in_=cps)
        nc.sync.dma_start(out=ctx_out[b:b + 1, :], in_=ctx_row)

    for pool in (row_pool, th_pool, eT_pool, ebf_pool, e32_pool,
                 ctx_pool, eps_pool, kp_pool, tp_pool, singles):
        pool.release()


def build_nc(debug=False) -> bass.Bass:
    nc = bacc.Bacc(None)
    dh = nc.declare_dram_parameter("dh", [BLOC, HID], F32, isOutput=False)
    enc = nc.declare_dram_parameter("enc", [BLOC, S, ENC], F32, isOutput=False)
    mask = nc.declare_dram_parameter("mask", [BLOC, S], I32, isOutput=False)
    wq = nc.declare_dram_parameter("wq", [HID, HID], F32, isOutput=False)
    wk = nc.declare_dram_parameter("wk", [HID, ENC], F32, isOutput=False)
    vv = nc.declare_dram_parameter("v", [HID], F32, isOutput=False)
    ctx_out = nc.declare_dram_parameter("ctx", [BLOC, ENC], F32, isOutput=True)
    attn_out = nc.declare_dram_parameter("attn", [BLOC, S], F32, isOutput=True)
    dbg = None
    if debug:
        dbg = {
            "q_all": nc.declare_dram_parameter("dbg_q", [P, HT, BLOC], F32,
                                               isOutput=True),
            "energy": nc.declare_dram_parameter("dbg_energy", [BLOC, S], F32,
                                                isOutput=True),
            "sumexp": nc.declare_dram_parameter("dbg_sumexp", [BLOC, 2], F32,
                                                isOutput=True),
            "th": nc.declare_dram_parameter("dbg_th", [P, SC, HT, 512], BF16,
                                            isOutput=True),
        }

    with tile.TileContext(nc) as tc:
        _emit(tc, nc, dh, enc, mask, wq, wk, vv, ctx_out, attn_out, dbg=dbg)
    nc.compile()
    return nc


def make_in_maps(decoder_hidden, encoder_outputs, mask, Wq, Wk, v):
    decoder_hidden = np.asarray(decoder_hidden, dtype=np.float32)
    encoder_outputs = np.asarray(encoder_outputs, dtype=np.float32)
    mask = np.asarray(mask, dtype=np.int32)
    Wq = np.asarray(Wq, dtype=np.float32)
    Wk = np.asarray(Wk, dtype=np.float32)
    v = np.asarray(v, dtype=np.float32)
    in_maps = []
    for c in range(NCORES):
        sl = slice(c * BLOC, (c + 1) * BLOC)
        in_maps.append({
            "dh": np.ascontiguousarray(decoder_hidden[sl]),
            "enc": np.ascontiguousarray(encoder_outputs[sl]),
            "mask": np.ascontiguousarray(mask[sl]),
            "wq": Wq, "wk": Wk, "v": v,
        })
    return in_maps


def kernel(decoder_hidden, encoder_outputs, mask, Wq, Wk, v):
    from concourse.bass_utils import run_bass_kernel_spmd
    nc = build_nc()
    in_maps = make_in_maps(decoder_hidden, encoder_outputs, mask, Wq, Wk, v)
    res = run_bass_kernel_spmd(nc, in_maps, core_ids=list(range(NCORES)))
    ctx = np.concatenate([r["ctx"] for r in res.results], axis=0)
    attn = np.concatenate([r["attn"] for r in res.results], axis=0)
    return ctx.astype(np.float32), attn.astype(np.float32)
